# revision 1
# baseline (speedup 1.0000x reference)
"""BiLSTM tagger Trainium2 kernel.

Sharding: data-parallel over batch (B=64 -> 8 per core). All weights
replicated. Each core computes tags for its 8 sentences; host gathers.

Device layout convention: "columns" j = s*BL + b  (word index within core).
Most tensors live transposed (features on partitions, words on free dim).

Per-core pipeline:
  1. char one-hot build -> ce.T = w_char.T @ onehot      [64, W*NW]
  2. char BiLSTM (batch=NW words), gates on partitions   -> hT_c[dir] [128, NW]
  3. word-emb gather (indirect DMA) + PE transpose       -> wembT [2][128, NW]
  4. embeds.T = w_final @ concat(...) + b                [2][128, NW]
  5. l1 input projection -> DRAM g1in [2, NW, 2048]  (rows = words, free = gates)
  6. l1 recurrence (batch rows on partitions, h-stationary matmuls),
     h transposed back via PE -> o1T [dir][128, 4, NW]
  7. l2 input projection from o1T -> g2in; l2 recurrence -> o2T
  8. tags.T = w_tag @ o2.T + b_tag -> DRAM [50, NW]
"""

import sys

sys.path.insert(0, "/opt/trn_rl_repo")

import numpy as np
import ml_dtypes

import concourse.bass as bass
import concourse.mybir as mybir
from concourse.bass import IndirectOffsetOnAxis
from concourse.tile import TileContext
from concourse.bass_utils import run_bass_kernel_spmd

F32 = mybir.dt.float32
BF16 = mybir.dt.bfloat16
I32 = mybir.dt.int32
SIG = mybir.ActivationFunctionType.Sigmoid
TANH = mybir.ActivationFunctionType.Tanh
MULT = mybir.AluOpType.mult
ADD = mybir.AluOpType.add
ISEQ = mybir.AluOpType.is_equal

NCORES = 8
BL = 8          # batch per core

LAST_RESULTS = {}  # stash for test harness (exec time etc.)


def split_multi_waits(nc, exempt=()):
    """Walrus here encodes at most one sync-wait per compute instruction;
    hoist extra waits onto standalone EventSemaphore ops on the same engine."""
    nsplit = 0
    for blk in nc.m.functions[0].blocks:
        newlist = []
        for inst in blk.instructions:
            si = inst.sync_info
            if si is not None and si.on_wait and len(si.on_wait) > 1 \
                    and inst.opcode not in exempt:
                waits = list(si.on_wait)
                for w in waits[:-1]:
                    ev = mybir.InstEventSemaphore(
                        name=f"{inst.name}-w{nsplit}", ins=[], outs=[],
                        sync_info=mybir.SyncInfo(on_wait=[w], on_update=[]),
                    )
                    ev.engine = inst.engine
                    newlist.append(ev)
                    nsplit += 1
                inst.sync_info = mybir.SyncInfo(
                    on_wait=[waits[-1]], on_update=list(si.on_update))
            newlist.append(inst)
        blk.instructions = newlist
    return nsplit


def build_nc(S=256, W=16, V=50000, CV=100, E=256, CD=64, H=512, T=50):
    NW = BL * S          # words per core
    NCH = NW * W         # chars per core
    G1 = 4 * H           # 2048 gates
    HC = E // 2          # char hidden = 128
    GC = 4 * HC          # 512 char gates
    NT = NW // 128       # word row tiles (16)
    NB = NW // 512       # 512-wide col chunks (4)

    nc = bass.Bass()
    def dp(name, shape, dtype, isOutput=False):
        return nc.declare_dram_parameter(name, shape, dtype, isOutput)

    widx = dp("widx", [128, NT], I32)
    cidx = dp("cidx", [1, NCH], F32)
    iota = dp("iota", [CV, 1], F32)
    idf = dp("idf", [128, 128], F32)
    idb = dp("idb", [128, 128], BF16)
    ones = dp("ones", [1, 128], BF16)
    onesf = dp("onesf", [1, 128], F32)
    wemb = dp("wemb", [V, E], F32)
    wch = dp("wch", [CV, CD], BF16)          # row 0 zeroed
    wihcT = dp("wihcT", [CD, 2 * GC], BF16)  # [64, dir*512+g]
    whhcT = dp("whhcT", [HC, 2 * GC], BF16)
    bc = dp("bc", [HC, 8], F32)              # [128, dir*4+gate]
    wfinT = dp("wfinT", [2 * E, E], BF16)
    bfin = dp("bfin", [E, 1], F32)
    wih1T = dp("wih1T", [2, E, G1], BF16)
    whh1T = dp("whh1T", [2, H, G1], BF16)
    b1 = dp("b1", [2, 1, G1], BF16)
    wih2T = dp("wih2T", [2, 2 * H, G1], BF16)
    whh2T = dp("whh2T", [2, H, G1], BF16)
    b2 = dp("b2", [2, 1, G1], BF16)
    wtagT = dp("wtagT", [2 * H, T], BF16)
    btag = dp("btag", [T, 1], F32)
    tags = dp("tags", [T, NW], F32, isOutput=True)

    g1in = nc.dram_tensor("g1in", [2, NW, G1], F32)
    g2in = nc.dram_tensor("g2in", [2, NW, G1], F32)

    with TileContext(nc) as tc:
        # ---------- persistent tiles ----------
        with tc.tile_pool(name="persist", bufs=1) as pp:
            idb_sb = pp.tile([128, 128], BF16)
            nc.sync.dma_start(out=idb_sb[:], in_=idb[:])
            ones_sb = pp.tile([1, 128], BF16)
            nc.sync.dma_start(out=ones_sb[:], in_=ones[:])
            onesf_sb = pp.tile([1, 128], F32)
            nc.sync.dma_start(out=onesf_sb[:], in_=onesf[:])
            embT = [pp.tile([128, NW], BF16, name=f"embT{m}") for m in range(2)]
            o1T = {d: pp.tile([128, 4, NW], BF16, name=f"o1T{d}") for d in "fb"}
            hT_c = {d: pp.tile([HC, NW], BF16, name=f"hTc{d}") for d in "fb"}
            wembT = [pp.tile([128, NW], BF16, name=f"wembT{m}") for m in range(2)]

            # ================= phase A: char LSTM + wemb gather =================
            with tc.tile_pool(name="phA", bufs=1) as pa, \
                 tc.tile_pool(name="phA_oh", bufs=3) as poh, \
                 tc.tile_pool(name="phA_ps", bufs=2, space="PSUM") as pps, \
                 tc.tile_pool(name="phA_cps", bufs=1, space="PSUM") as cps, \
                 tc.tile_pool(name="phA_tmp", bufs=2) as ptmp, \
                 tc.tile_pool(name="phA_g", bufs=3) as pgather, \
                 tc.tile_pool(name="phA_tps", bufs=2, space="PSUM") as ptps:
                iota_sb = pa.tile([CV, 1], F32)
                nc.sync.dma_start(out=iota_sb[:], in_=iota[:])
                idf_sb = pa.tile([128, 128], F32)
                nc.sync.dma_start(out=idf_sb[:], in_=idf[:])
                wch_sb = pa.tile([CV, CD], BF16)
                nc.sync.dma_start(out=wch_sb[:], in_=wch[:])
                wihc_sb = pa.tile([CD, 2 * GC], BF16)
                nc.sync.dma_start(out=wihc_sb[:], in_=wihcT[:])
                whhc_sb = pa.tile([HC, 2 * GC], BF16)
                nc.sync.dma_start(out=whhc_sb[:], in_=whhcT[:])
                bc_sb = pa.tile([HC, 8], F32)
                nc.sync.dma_start(out=bc_sb[:], in_=bc[:])
                widx_sb = pa.tile([128, NT], I32)
                nc.sync.dma_start(out=widx_sb[:], in_=widx[:])
                ceT = pa.tile([CD, NCH], BF16)
                c_c = {d: pa.tile([HC, NW], F32, name=f"cc{d}") for d in "fb"}

                # --- ce.T via one-hot matmul, 512-wide chunks ---
                for n in range(NCH // 512):
                    cch = poh.tile([1, 512], F32, tag="cch")
                    nc.sync.dma_start(out=cch[:],
                                      in_=cidx[0:1, n * 512:(n + 1) * 512])
                    pbc = pps.tile([CV, 512], F32, tag="pbc", bufs=1)
                    nc.tensor.matmul(pbc[:], onesf_sb[0:1, 0:CV], cch[:],
                                     start=True, stop=True)
                    oh = poh.tile([CV, 512], BF16, tag="oh")
                    nc.vector.tensor_tensor(
                        out=oh[:],
                        in0=iota_sb[:].to_broadcast([CV, 512]),
                        in1=pbc[:],
                        op=ISEQ,
                    )
                    pce = pps.tile([CD, 512], F32, tag="pce", bufs=1)
                    nc.tensor.matmul(pce[:], wch_sb[:], oh[:], start=True, stop=True)
                    nc.vector.tensor_copy(
                        out=ceT[:, n * 512:(n + 1) * 512], in_=pce[:]
                    )

                # --- word-emb gather + transpose (overlaps char compute) ---
                for t in range(NT):
                    wg = pgather.tile([128, E], F32, tag="wg")
                    nc.gpsimd.indirect_dma_start(
                        out=wg[:],
                        out_offset=None,
                        in_=wemb[:],
                        in_offset=IndirectOffsetOnAxis(ap=widx_sb[:, t:t + 1], axis=0),
                    )
                    for m in range(E // 128):
                        ptw = ptps.tile([128, 128], F32, tag="ptw")
                        nc.tensor.transpose(
                            out=ptw[:],
                            in_=wg[:, m * 128:(m + 1) * 128],
                            identity=idf_sb[:],
                        )
                        nc.vector.tensor_copy(
                            out=wembT[m][:, t * 128:(t + 1) * 128], in_=ptw[:]
                        )

                # --- char recurrence ---
                for it in range(W):
                    for d in "fb":
                        w = it if d == "f" else W - 1 - it
                        first = it == 0
                        do = 0 if d == "f" else GC
                        for n in range(NB):
                            cs = slice(n * 512, (n + 1) * 512)
                            pg = [cps.tile([HC, 512], F32, tag=f"pg{m}", name=f"pg{m}")
                                  for m in range(4)]
                            for m in range(4):
                                nc.tensor.matmul(
                                    pg[m][:],
                                    wihc_sb[:, do + m * 128:do + (m + 1) * 128],
                                    ceT[:, w * NW + n * 512:w * NW + (n + 1) * 512],
                                    start=True, stop=first,
                                )
                                if not first:
                                    nc.tensor.matmul(
                                        pg[m][:],
                                        whhc_sb[:, do + m * 128:do + (m + 1) * 128],
                                        hT_c[d][:, cs],
                                        start=False, stop=True,
                                    )
                            gd = 0 if d == "f" else 4
                            t_i = ptmp.tile([HC, 512], F32, tag="ti")
                            t_f = ptmp.tile([HC, 512], F32, tag="tf")
                            t_g = ptmp.tile([HC, 512], F32, tag="tg")
                            t_o = ptmp.tile([HC, 512], F32, tag="to")
                            nc.scalar.activation(t_i[:], pg[0][:], SIG,
                                                 bias=bc_sb[:, gd + 0:gd + 1])
                            nc.scalar.activation(t_f[:], pg[1][:], SIG,
                                                 bias=bc_sb[:, gd + 1:gd + 2])
                            nc.scalar.activation(t_g[:], pg[2][:], TANH,
                                                 bias=bc_sb[:, gd + 2:gd + 3])
                            nc.scalar.activation(t_o[:], pg[3][:], SIG,
                                                 bias=bc_sb[:, gd + 3:gd + 4])
                            if first:
                                nc.vector.tensor_tensor(
                                    out=c_c[d][:, cs], in0=t_i[:], in1=t_g[:], op=MULT)
                            else:
                                nc.vector.tensor_tensor(
                                    out=t_f[:], in0=t_f[:], in1=c_c[d][:, cs], op=MULT)
                                nc.vector.tensor_tensor(
                                    out=t_i[:], in0=t_i[:], in1=t_g[:], op=MULT)
                                nc.vector.tensor_tensor(
                                    out=c_c[d][:, cs], in0=t_f[:], in1=t_i[:], op=ADD)
                            nc.scalar.activation(t_g[:], c_c[d][:, cs], TANH)
                            nc.vector.tensor_tensor(
                                out=hT_c[d][:, cs], in0=t_o[:], in1=t_g[:], op=MULT)

            # ================= phase B: embeds + l1 projection =================
            with tc.tile_pool(name="phB", bufs=1) as pb, \
                 tc.tile_pool(name="phB_ps", bufs=1, space="PSUM") as bps, \
                 tc.tile_pool(name="phB_st", bufs=4) as bst:
                wfin0 = pb.tile([128, E], BF16)
                nc.sync.dma_start(out=wfin0[:], in_=wfinT[0:128, :])
                wfin1 = pb.tile([128, E], BF16)
                nc.sync.dma_start(out=wfin1[:], in_=wfinT[128:256, :])
                wfin2 = pb.tile([128, E], BF16)
                nc.sync.dma_start(out=wfin2[:], in_=wfinT[256:384, :])
                wfin3 = pb.tile([128, E], BF16)
                nc.sync.dma_start(out=wfin3[:], in_=wfinT[384:512, :])
                wfk = [wfin0, wfin1, wfin2, wfin3]
                bfin_sb = [pb.tile([128, 1], F32, name=f"bf{m}") for m in range(2)]
                for m in range(2):
                    nc.sync.dma_start(out=bfin_sb[m][:],
                                      in_=bfin[m * 128:(m + 1) * 128, :])
                xk = [wembT[0], wembT[1], hT_c["f"], hT_c["b"]]
                for m in range(2):
                    for n in range(NB):
                        cs = slice(n * 512, (n + 1) * 512)
                        pe = bps.tile([128, 512], F32, tag="pe", bufs=2)
                        for k in range(4):
                            nc.tensor.matmul(
                                pe[:], wfk[k][:, m * 128:(m + 1) * 128],
                                xk[k][:, cs], start=(k == 0), stop=(k == 3))
                        nc.vector.tensor_scalar(
                            out=embT[m][:, cs], in0=pe[:],
                            scalar1=bfin_sb[m][:], scalar2=None, op0=ADD)

                # l1 projection -> g1in
                wih1_sb = {}
                b1_sb = {}
                for di, d in enumerate("fb"):
                    b1_sb[d] = pb.tile([1, G1], BF16, name=f"b1{d}")
                    nc.sync.dma_start(out=b1_sb[d][:], in_=b1[di, :, :])
                    for k in range(2):
                        tl = pb.tile([128, G1], BF16, name=f"wih1{d}{k}")
                        nc.sync.dma_start(out=tl[:],
                                          in_=wih1T[di, k * 128:(k + 1) * 128, :])
                        wih1_sb[d, k] = tl
                for di, d in enumerate("fb"):
                    for r in range(NT):
                        rs = slice(r * 128, (r + 1) * 128)
                        ppn = [bps.tile([128, 512], F32, tag=f"pp{n}", name=f"pp{n}")
                               for n in range(4)]
                        for k in range(2):
                            for n in range(4):
                                nc.tensor.matmul(
                                    ppn[n][:], embT[k][:, rs],
                                    wih1_sb[d, k][:, n * 512:(n + 1) * 512],
                                    start=(k == 0), stop=False)
                        for n in range(4):
                            nc.tensor.matmul(
                                ppn[n][:], ones_sb[:],
                                b1_sb[d][0:1, n * 512:(n + 1) * 512],
                                start=False, stop=True)
                        for n in range(4):
                            st = bst.tile([128, 512], F32, tag="st")
                            nc.vector.tensor_copy(out=st[:], in_=ppn[n][:])
                            nc.sync.dma_start(
                                out=g1in[di, rs, n * 512:(n + 1) * 512], in_=st[:])

            # ================= phase C: l1 recurrence =================
            def recurrence(layer, gin, whhT_dram, hist):
                with tc.tile_pool(name=f"rc{layer}", bufs=1) as pc, \
                     tc.tile_pool(name=f"rc{layer}_gi", bufs=2) as pgi, \
                     tc.tile_pool(name=f"rc{layer}_ps", bufs=3, space="PSUM") as wps, \
                     tc.tile_pool(name=f"rc{layer}_tr", bufs=2, space="PSUM") as tps, \
                     tc.tile_pool(name=f"rc{layer}_tmp", bufs=2) as ptm:
                    whh_sb = {}
                    for di, d in enumerate("fb"):
                        for k in range(4):
                            tl = pc.tile([128, G1], BF16, name=f"whh{layer}{d}{k}")
                            nc.sync.dma_start(
                                out=tl[:], in_=whhT_dram[di, k * 128:(k + 1) * 128, :])
                            whh_sb[d, k] = tl
                    c_w = {d: pc.tile([BL, H], F32, name=f"cw{layer}{d}")
                           for d in "fb"}
                    for it in range(S):
                        for d in "fb":
                            s = it if d == "f" else S - 1 - it
                            sp = s - 1 if d == "f" else s + 1
                            first = it == 0
                            di = 0 if d == "f" else 1
                            g_in = pgi.tile([BL, G1], F32, tag="gin")
                            nc.sync.dma_start(
                                out=g_in[:],
                                in_=gin[di, s * BL:(s + 1) * BL, :])
                            if not first:
                                pgA = wps.tile([BL, 1024], F32, tag="pg")
                                pgB = wps.tile([BL, 1024], F32, tag="pg")
                                for k in range(4):
                                    lhs = hist[d][:, k, sp * BL:(sp + 1) * BL]
                                    for n in range(4):
                                        pt = pgA if n < 2 else pgB
                                        nc.tensor.matmul(
                                            pt[:, (n % 2) * 512:(n % 2 + 1) * 512],
                                            lhs,
                                            whh_sb[d, k][:, n * 512:(n + 1) * 512],
                                            start=(k == 0), stop=(k == 3))
                                nc.vector.tensor_tensor(
                                    out=g_in[:, 0:1024], in0=pgA[:],
                                    in1=g_in[:, 0:1024], op=ADD)
                                nc.vector.tensor_tensor(
                                    out=g_in[:, 1024:2048], in0=pgB[:],
                                    in1=g_in[:, 1024:2048], op=ADD)
                                gap = g_in
                            else:
                                gap = g_in
                            t_if = ptm.tile([BL, 1024], F32, tag="tif")
                            t_g = ptm.tile([BL, H], F32, tag="tg2")
                            t_o = ptm.tile([BL, H], F32, tag="to2")
                            nc.scalar.activation(t_if[:], gap[:, 0:1024], SIG)
                            nc.scalar.activation(t_g[:], gap[:, 1024:1536], TANH)
                            nc.scalar.activation(t_o[:], gap[:, 1536:2048], SIG)
                            if first:
                                nc.vector.tensor_tensor(
                                    out=c_w[d][:], in0=t_if[:, 0:512],
                                    in1=t_g[:], op=MULT)
                            else:
                                nc.vector.tensor_tensor(
                                    out=t_if[:, 512:1024], in0=t_if[:, 512:1024],
                                    in1=c_w[d][:], op=MULT)
                                nc.vector.tensor_tensor(
                                    out=t_if[:, 0:512], in0=t_if[:, 0:512],
                                    in1=t_g[:], op=MULT)
                                nc.vector.tensor_tensor(
                                    out=c_w[d][:], in0=t_if[:, 512:1024],
                                    in1=t_if[:, 0:512], op=ADD)
                            nc.scalar.activation(t_g[:], c_w[d][:], TANH)
                            h_row = ptm.tile([BL, H], BF16, tag="hrow")
                            nc.vector.tensor_tensor(
                                out=h_row[:], in0=t_o[:], in1=t_g[:], op=MULT)
                            ptr = tps.tile([128, 32], BF16, tag="ptr")
                            for k in range(4):
                                nc.tensor.transpose(
                                    out=ptr[:, k * 8:(k + 1) * 8],
                                    in_=h_row[:, k * 128:(k + 1) * 128],
                                    identity=idb_sb[0:BL, 0:BL])
                            nc.vector.tensor_copy(
                                out=hist[d][:, :, s * BL:(s + 1) * BL],
                                in_=ptr[:].rearrange("p (k c) -> p k c", k=4))

            recurrence(1, g1in, whh1T, o1T)

            # ================= phase D: l2 projection =================
            o2T = {d: pp.tile([128, 4, NW], BF16, name=f"o2T{d}") for d in "fb"}
            with tc.tile_pool(name="phD", bufs=1) as pd_, \
                 tc.tile_pool(name="phD_ps", bufs=2, space="PSUM") as dps, \
                 tc.tile_pool(name="phD_st", bufs=6) as dst:
                wih2_sb = {}
                b2_sb = {}
                for di, d in enumerate("fb"):
                    b2_sb[d] = pd_.tile([1, G1], BF16, name=f"b2{d}")
                    nc.sync.dma_start(out=b2_sb[d][:], in_=b2[di, :, :])
                    for k in range(8):
                        tl = pd_.tile([128, G1], BF16, name=f"wih2{d}{k}")
                        nc.sync.dma_start(out=tl[:],
                                          in_=wih2T[di, k * 128:(k + 1) * 128, :])
                        wih2_sb[d, k] = tl
                for di, d in enumerate("fb"):
                    for r in range(NT):
                        rs = slice(r * 128, (r + 1) * 128)
                        ppn = [dps.tile([128, 512], F32, tag=f"qq{n}", name=f"qq{n}")
                               for n in range(4)]
                        for k in range(8):
                            src = o1T["f"] if k < 4 else o1T["b"]
                            lhs = src[:, k % 4, rs]
                            for n in range(4):
                                nc.tensor.matmul(
                                    ppn[n][:], lhs,
                                    wih2_sb[d, k][:, n * 512:(n + 1) * 512],
                                    start=(k == 0), stop=False)
                        for n in range(4):
                            nc.tensor.matmul(
                                ppn[n][:], ones_sb[:],
                                b2_sb[d][0:1, n * 512:(n + 1) * 512],
                                start=False, stop=True)
                        for n in range(4):
                            st = dst.tile([128, 512], F32, tag="st2")
                            nc.vector.tensor_copy(out=st[:], in_=ppn[n][:])
                            nc.sync.dma_start(
                                out=g2in[di, rs, n * 512:(n + 1) * 512], in_=st[:])

            # ================= phase E: l2 recurrence =================
            recurrence(2, g2in, whh2T, o2T)

            # ================= phase F: tag projection =================
            with tc.tile_pool(name="phF", bufs=1) as pf, \
                 tc.tile_pool(name="phF_ps", bufs=4, space="PSUM") as fps, \
                 tc.tile_pool(name="phF_st", bufs=4) as fst:
                wtag_sb = pf.tile([128, 8 * T], BF16)
                for k in range(8):
                    nc.sync.dma_start(
                        out=wtag_sb[:, k * T:(k + 1) * T],
                        in_=wtagT[k * 128:(k + 1) * 128, :])
                btag_sb = pf.tile([T, 1], F32)
                nc.sync.dma_start(out=btag_sb[:], in_=btag[:])
                for n in range(NB):
                    cs = slice(n * 512, (n + 1) * 512)
                    pt = fps.tile([T, 512], F32, tag="pt")
                    for k in range(8):
                        src = o2T["f"] if k < 4 else o2T["b"]
                        nc.tensor.matmul(
                            pt[:], wtag_sb[:, k * T:(k + 1) * T],
                            src[:, k % 4, cs],
                            start=(k == 0), stop=(k == 7))
                    st = fst.tile([T, 512], F32, tag="st3")
                    nc.vector.tensor_scalar(
                        out=st[:], in0=pt[:], scalar1=btag_sb[:],
                        scalar2=None, op0=ADD)
                    nc.sync.dma_start(out=tags[:, cs], in_=st[:])

    split_multi_waits(nc)
    return nc


def prep_inputs(inputs, S=256, W=16, V=50000, CV=100, E=256, CD=64, H=512, T=50):
    """Host-side: shard + transpose/cast weights. Returns in_maps list."""
    f32 = np.float32
    bf16 = ml_dtypes.bfloat16
    NW = BL * S
    NT = NW // 128

    def p(name):
        return np.asarray(inputs[name])

    sent = p("sentence").astype(np.int32)           # [B, S]
    csent = p("char_sentence").astype(np.int32)     # [B, S, W]

    wchz = p("w_char").astype(f32).copy()
    wchz[0] = 0.0

    common = {
        "iota": np.arange(CV, dtype=f32).reshape(CV, 1),
        "idf": np.eye(128, dtype=f32),
        "idb": np.eye(128).astype(bf16),
        "ones": np.ones((1, 128), dtype=bf16),
        "onesf": np.ones((1, 128), dtype=f32),
        "wemb": p("w_emb").astype(f32),
        "wch": wchz.astype(bf16),
        "wihcT": np.concatenate(
            [p("cf_wih").T, p("cb_wih").T], axis=1).astype(bf16),
        "whhcT": np.concatenate(
            [p("cf_whh").T, p("cb_whh").T], axis=1).astype(bf16),
        "bc": np.stack(
            [(p(f"{c}_bih") + p(f"{c}_bhh")).astype(f32).reshape(4, E // 2)[g]
             for c in ("cf", "cb") for g in range(4)], axis=1),
        "wfinT": p("w_final").T.astype(bf16),
        "bfin": p("b_final").astype(f32).reshape(E, 1),
        "wih1T": np.stack([p("l1f_wih").T, p("l1b_wih").T]).astype(bf16),
        "whh1T": np.stack([p("l1f_whh").T, p("l1b_whh").T]).astype(bf16),
        "b1": np.stack([(p("l1f_bih") + p("l1f_bhh")).reshape(1, 4 * H),
                        (p("l1b_bih") + p("l1b_bhh")).reshape(1, 4 * H)]
                       ).astype(bf16),
        "wih2T": np.stack([p("l2f_wih").T, p("l2b_wih").T]).astype(bf16),
        "whh2T": np.stack([p("l2f_whh").T, p("l2b_whh").T]).astype(bf16),
        "b2": np.stack([(p("l2f_bih") + p("l2f_bhh")).reshape(1, 4 * H),
                        (p("l2b_bih") + p("l2b_bhh")).reshape(1, 4 * H)]
                       ).astype(bf16),
        "wtagT": p("w_tag").T.astype(bf16),
        "btag": p("b_tag").astype(f32).reshape(T, 1),
    }

    in_maps = []
    for c in range(NCORES):
        sl = sent[c * BL:(c + 1) * BL]              # [BL, S]
        cl = csent[c * BL:(c + 1) * BL]             # [BL, S, W]
        # word col j = s*BL + b
        wflat = sl.T.reshape(NW)                    # [(s b)]
        widx = wflat.reshape(NT, 128).T.astype(np.int32).copy()  # [128, NT]
        cflat = cl.transpose(2, 1, 0).reshape(1, W * NW).astype(f32)
        m = dict(common)
        m["widx"] = np.ascontiguousarray(widx)
        m["cidx"] = np.ascontiguousarray(cflat)
        in_maps.append(m)
    return in_maps


def unshard(results, S=256, T=50):
    NW = BL * S
    out = np.empty((NCORES * BL, S, T), dtype=np.float32)
    for c, r in enumerate(results):
        tg = r["tags"]                              # [T, NW], col = s*BL+b
        out[c * BL:(c + 1) * BL] = tg.reshape(T, S, BL).transpose(2, 1, 0)
    return out


_NC_CACHE = {}


def kernel(**inputs):
    import os
    key = "default"
    if key not in _NC_CACHE:
        _NC_CACHE[key] = build_nc()
    nc = _NC_CACHE[key]
    in_maps = prep_inputs(inputs)
    trace = bool(int(os.environ.get("BK_TRACE", "0")))
    res = run_bass_kernel_spmd(nc, in_maps, core_ids=list(range(NCORES)),
                               trace=trace)
    LAST_RESULTS["res"] = res
    return unshard(res.results)

